# revision 1
# baseline (speedup 1.0000x reference)
"""Trainium2 Bass kernel for nn_Net_modes_50697793962433.

Computes out = tanh(einsum('hrk,bkr->bh', W, x[:,0])) @ V.T + b for
x [8192,1,512,16], W [512,16,512], V [16,512], b [16] -> out [8192,16].

Sharding: data-parallel over batch across 8 NeuronCores; W/V/b replicated.
Host prep per core c: xt = x.reshape(8192,8192)[c*1024:(c+1)*1024].T (fp16),
wt = W.transpose(2,1,0).reshape(8192,512) (fp16) so the contraction dim
(k*16+r) lands on SBUF partitions with fully contiguous DMAs.

Device per core: s^T[h,j] accumulates in 8 PSUM banks (4 h-chunks x 2
j-chunks of 512) over 64 k-tiles of fp16 matmuls, issued in bank-stable
runs per k-group (ramped group sizes so the PE starts early); tanh on
ScalarE straight out of PSUM into fp16 h^T tiles; V^T-stationary matmul
produces outT [16,1024] in PSUM; DVE adds bias; one contiguous outT store
per core and the host transposes back.

fp16 matmul keeps 11 mantissa bits (rel err ~4e-4 vs fp32 reference,
~6x better than bf16) while halving HBM traffic and running the PE at
1 cycle/row.
"""
import numpy as np
import concourse.bacc as bacc
import concourse.mybir as mybir
import concourse.tile as tile
from concourse.bass_utils import run_bass_kernel_spmd

N_CORES = 8
B, HID, R, K, NCLS = 8192, 512, 16, 512, 16
KR = K * R            # 8192 contraction length
BL = B // N_CORES     # 1024 batch rows per core
NKT = KR // 128       # 64 k-tiles
NHC = HID // 128      # 4 h-chunks
NJC = BL // 512       # 2 j-chunks

F32 = mybir.dt.float32
F16 = mybir.dt.float16

XBUFS = 24
WBUFS = 24
# ramped k-groups: small first groups so the PE starts as soon as the
# first tiles land; 8-tile groups amortize PSUM bank switches after that
GROUPS = [2, 2, 4] + [8] * ((NKT - 8) // 8)


def _build_nc():
    nc = bacc.Bacc("TRN2", target_bir_lowering=False, debug=False,
                   num_devices=N_CORES)
    xt_d = nc.dram_tensor("xt", [KR, BL], F16, kind="ExternalInput")
    wt_d = nc.dram_tensor("wt", [KR, HID], F16, kind="ExternalInput")
    vt_d = nc.dram_tensor("vt", [HID, NCLS], F16, kind="ExternalInput")
    bias_d = nc.dram_tensor("bias", [NCLS, 1], F32, kind="ExternalInput")
    out_d = nc.dram_tensor("outT", [NCLS, BL], F32, kind="ExternalOutput")

    xt_v = xt_d.ap().rearrange("(n p) b -> n p b", p=128)    # [64,128,1024]
    wt_v = wt_d.ap().rearrange("(n p) h -> n p h", p=128)    # [64,128,512]
    vt_v = vt_d.ap().rearrange("(n p) c -> n p c", p=128)    # [4,128,16]

    with tile.TileContext(nc) as tc:
        with (
            tc.tile_pool(name="xin", bufs=XBUFS) as xpool,
            tc.tile_pool(name="win", bufs=WBUFS) as wpool,
            tc.tile_pool(name="hbuf", bufs=NHC * NJC) as hpool,
            tc.tile_pool(name="const", bufs=1) as cpool,
            tc.tile_pool(name="obuf", bufs=1) as opool,
            tc.tile_pool(name="psum", bufs=8, space="PSUM") as pspool,
        ):
            vt_sb = cpool.tile([128, NHC * NCLS], F16, tag="vt")
            for hc in range(NHC):
                nc.sync.dma_start(vt_sb[:, hc * NCLS:(hc + 1) * NCLS], vt_v[hc])
            bias_sb = cpool.tile([NCLS, 1], F32, tag="bias")
            nc.sync.dma_start(bias_sb[:], bias_d.ap()[:])

            acc = [pspool.tile([128, 512], F32, tag="acc", name=f"acc{i}")
                   for i in range(NHC * NJC)]
            kg = 0
            for g in GROUPS:
                xts, wts = [], []
                for k in range(kg, kg + g):
                    xtile = xpool.tile([128, BL], F16, tag="x", name="xtile")
                    nc.sync.dma_start(xtile[:], xt_v[k])
                    wtile = wpool.tile([128, HID], F16, tag="w", name="wtile")
                    nc.sync.dma_start(wtile[:], wt_v[k])
                    xts.append(xtile)
                    wts.append(wtile)
                # bank-stable runs: per (hc, jc) PSUM bank, g consecutive
                # accumulating matmuls
                for hc in range(NHC):
                    for jc in range(NJC):
                        for i, k in enumerate(range(kg, kg + g)):
                            nc.tensor.matmul(
                                acc[hc * NJC + jc][:],
                                wts[i][:, hc * 128:(hc + 1) * 128],
                                xts[i][:, jc * 512:(jc + 1) * 512],
                                start=(k == 0), stop=(k == NKT - 1),
                            )
                kg += g

            # tanh(s^T) -> fp16 h^T tiles; jc-major so the jc=0 output
            # matmul overlaps the jc=1 tanh
            hsb = [[hpool.tile([128, 512], F16, tag="h", name=f"h{hc}_{jc}")
                    for jc in range(NJC)] for hc in range(NHC)]
            for jc in range(NJC):
                for hc in range(NHC):
                    nc.scalar.activation(
                        hsb[hc][jc][:],
                        acc[hc * NJC + jc][:],
                        mybir.ActivationFunctionType.Tanh,
                    )
            # outT[c, j] = sum_h vt[h, c] * h^T[h, j]  (+ bias)
            outT = opool.tile([NCLS, BL], F32, tag="o", name="outT")
            for jc in range(NJC):
                ps2 = pspool.tile([NCLS, 512], F32, tag="acc", name="ps2")
                for hc in range(NHC):
                    nc.tensor.matmul(
                        ps2[:],
                        vt_sb[:, hc * NCLS:(hc + 1) * NCLS],
                        hsb[hc][jc][:],
                        start=(hc == 0), stop=(hc == NHC - 1),
                    )
                nc.vector.tensor_scalar_add(
                    outT[:, jc * 512:(jc + 1) * 512], ps2[:], bias_sb[:])
            nc.sync.dma_start(out_d.ap()[:], outT[:])
    nc.compile()
    return nc


_NC_CACHE = None


def kernel(x, W, V, b):
    global _NC_CACHE
    x = np.asarray(x, dtype=np.float32)
    W = np.asarray(W, dtype=np.float32)
    V = np.asarray(V, dtype=np.float32)
    b = np.asarray(b, dtype=np.float32)

    wt = np.ascontiguousarray(W.transpose(2, 1, 0).reshape(KR, HID),
                              dtype=np.float16)
    vt = np.ascontiguousarray(V.T, dtype=np.float16)
    bias = np.ascontiguousarray(b.reshape(NCLS, 1))
    xr = x.reshape(B, KR)
    in_maps = []
    for c in range(N_CORES):
        xt_c = np.ascontiguousarray(xr[c * BL:(c + 1) * BL].T,
                                    dtype=np.float16)
        in_maps.append({"xt": xt_c, "wt": wt, "vt": vt, "bias": bias})

    if _NC_CACHE is None:
        _NC_CACHE = _build_nc()
    res = run_bass_kernel_spmd(_NC_CACHE, in_maps,
                               core_ids=list(range(N_CORES)))
    if res.exec_time_ns is not None:
        print(f"HW exec time: {res.exec_time_ns} ns")
    return np.concatenate(
        [res.results[c]["outT"].T for c in range(N_CORES)], axis=0)


# revision 3
# speedup vs baseline: 1.0030x; 1.0030x over previous
"""Trainium2 Bass kernel for nn_Net_modes_50697793962433.

Computes out = tanh(einsum('hrk,bkr->bh', W, x[:,0])) @ V.T + b for
x [8192,1,512,16], W [512,16,512], V [16,512], b [16] -> out [8192,16].

Sharding: data-parallel over batch across 8 NeuronCores; W/V/b replicated.
Host prep per core c: xt = x.reshape(8192,8192)[c*1024:(c+1)*1024].T (fp16),
wt = W.transpose(2,1,0).reshape(8192,512) (fp16) so the contraction dim
(k*16+r) lands on SBUF partitions with fully contiguous DMAs.

Device per core: s^T[h,j] accumulates in 8 PSUM banks (4 h-chunks x 2
j-chunks of 512) over 64 k-tiles of fp16 matmuls, issued in bank-stable
runs per k-group (ramped group sizes so the PE starts early); tanh on
ScalarE straight out of PSUM into fp16 h^T tiles; V^T-stationary matmul
produces outT [16,1024] in PSUM; DVE adds bias; one contiguous outT store
per core and the host transposes back.

fp16 matmul keeps 11 mantissa bits (rel err ~4e-4 vs fp32 reference,
~6x better than bf16) while halving HBM traffic and running the PE at
1 cycle/row.
"""
import numpy as np
import concourse.bacc as bacc
import concourse.mybir as mybir
import concourse.tile as tile
from concourse.bass_utils import run_bass_kernel_spmd

N_CORES = 8
B, HID, R, K, NCLS = 8192, 512, 16, 512, 16
KR = K * R            # 8192 contraction length
BL = B // N_CORES     # 1024 batch rows per core
NKT = KR // 128       # 64 k-tiles
NHC = HID // 128      # 4 h-chunks
NJC = BL // 512       # 2 j-chunks

F32 = mybir.dt.float32
F16 = mybir.dt.float16

XBUFS = 24
WBUFS = 24
# ramped k-groups: small first groups so the PE starts as soon as the
# first tiles land; 8-tile groups amortize PSUM bank switches after that
GROUPS = [2, 2, 4] + [8] * ((NKT - 8) // 8)


def _build_nc():
    nc = bacc.Bacc("TRN2", target_bir_lowering=False, debug=False,
                   num_devices=N_CORES)
    xt_d = nc.dram_tensor("xt", [KR, BL], F16, kind="ExternalInput")
    wt_d = nc.dram_tensor("wt", [KR, HID], F16, kind="ExternalInput")
    vt_d = nc.dram_tensor("vt", [HID, NCLS], F16, kind="ExternalInput")
    bias_d = nc.dram_tensor("bias", [NCLS, 1], F32, kind="ExternalInput")
    out_d = nc.dram_tensor("outT", [NCLS, BL], F32, kind="ExternalOutput")

    xt_v = xt_d.ap().rearrange("(n p) b -> n p b", p=128)    # [64,128,1024]
    wt_v = wt_d.ap().rearrange("(n p) h -> n p h", p=128)    # [64,128,512]
    vt_v = vt_d.ap().rearrange("(n p) c -> n p c", p=128)    # [4,128,16]

    with tile.TileContext(nc) as tc:
        with (
            tc.tile_pool(name="xin", bufs=XBUFS) as xpool,
            tc.tile_pool(name="win", bufs=WBUFS) as wpool,
            tc.tile_pool(name="hbuf", bufs=NHC * NJC) as hpool,
            tc.tile_pool(name="const", bufs=1) as cpool,
            tc.tile_pool(name="obuf", bufs=1) as opool,
            tc.tile_pool(name="psum", bufs=8, space="PSUM") as pspool,
        ):
            # constants go on the ScalarE HWDGE ring so they never queue
            # ahead of the streaming x/w loads on the sync ring
            vt_sb = cpool.tile([128, NHC * NCLS], F16, tag="vt")
            for hc in range(NHC):
                nc.scalar.dma_start(vt_sb[:, hc * NCLS:(hc + 1) * NCLS],
                                    vt_v[hc])
            bias_sb = cpool.tile([NCLS, 1], F32, tag="bias")
            nc.scalar.dma_start(bias_sb[:], bias_d.ap()[:])

            acc = [pspool.tile([128, 512], F32, tag="acc", name=f"acc{i}")
                   for i in range(NHC * NJC)]
            kg = 0
            for g in GROUPS:
                xts, wts = [], []
                for k in range(kg, kg + g):
                    xtile = xpool.tile([128, BL], F16, tag="x", name="xtile")
                    nc.sync.dma_start(xtile[:], xt_v[k])
                    wtile = wpool.tile([128, HID], F16, tag="w", name="wtile")
                    nc.sync.dma_start(wtile[:], wt_v[k])
                    xts.append(xtile)
                    wts.append(wtile)
                # bank-stable runs: per (hc, jc) PSUM bank, g consecutive
                # accumulating matmuls. In the last group, run jc=1 banks
                # first so their tanh/MM2/store tail overlaps the
                # remaining jc=0 runs on the PE.
                last = kg + g == NKT
                runs = [(hc, jc) for jc in ((1, 0) if last else (0, 1))
                        for hc in (range(NHC - 1, -1, -1) if last
                                   else range(NHC))]
                for hc, jc in runs:
                    for i, k in enumerate(range(kg, kg + g)):
                        nc.tensor.matmul(
                            acc[hc * NJC + jc][:],
                            wts[i][:, hc * 128:(hc + 1) * 128],
                            xts[i][:, jc * 512:(jc + 1) * 512],
                            start=(k == 0), stop=(k == NKT - 1),
                        )
                kg += g

            # tanh(s^T) -> fp16 h^T tiles, then
            # outT[c, j] = sum_h vt[h, c] * h^T[h, j]  (+ bias);
            # jc=1 first (its banks stopped first in the last group) and
            # each half stored as soon as it is ready
            hsb = [[hpool.tile([128, 512], F16, tag="h", name=f"h{hc}_{jc}")
                    for jc in range(NJC)] for hc in range(NHC)]
            outT = opool.tile([NCLS, BL], F32, tag="o", name="outT")
            for jc in (1, 0):
                for hc in range(NHC):
                    nc.scalar.activation(
                        hsb[hc][jc][:],
                        acc[hc * NJC + jc][:],
                        mybir.ActivationFunctionType.Tanh,
                    )
                ps2 = pspool.tile([NCLS, 512], F32, tag="acc", name="ps2")
                for hc in range(NHC):
                    nc.tensor.matmul(
                        ps2[:],
                        vt_sb[:, hc * NCLS:(hc + 1) * NCLS],
                        hsb[hc][jc][:],
                        start=(hc == 0), stop=(hc == NHC - 1),
                    )
                nc.vector.tensor_scalar_add(
                    outT[:, jc * 512:(jc + 1) * 512], ps2[:], bias_sb[:])
                nc.sync.dma_start(
                    out_d.ap()[:, jc * 512:(jc + 1) * 512],
                    outT[:, jc * 512:(jc + 1) * 512])
    nc.compile()
    return nc


_NC_CACHE = None


def kernel(x, W, V, b):
    global _NC_CACHE
    x = np.asarray(x, dtype=np.float32)
    W = np.asarray(W, dtype=np.float32)
    V = np.asarray(V, dtype=np.float32)
    b = np.asarray(b, dtype=np.float32)

    wt = np.ascontiguousarray(W.transpose(2, 1, 0).reshape(KR, HID),
                              dtype=np.float16)
    vt = np.ascontiguousarray(V.T, dtype=np.float16)
    bias = np.ascontiguousarray(b.reshape(NCLS, 1))
    xr = x.reshape(B, KR)
    in_maps = []
    for c in range(N_CORES):
        xt_c = np.ascontiguousarray(xr[c * BL:(c + 1) * BL].T,
                                    dtype=np.float16)
        in_maps.append({"xt": xt_c, "wt": wt, "vt": vt, "bias": bias})

    if _NC_CACHE is None:
        _NC_CACHE = _build_nc()
    res = run_bass_kernel_spmd(_NC_CACHE, in_maps,
                               core_ids=list(range(N_CORES)))
    if res.exec_time_ns is not None:
        print(f"HW exec time: {res.exec_time_ns} ns")
    return np.concatenate(
        [res.results[c]["outT"].T for c in range(N_CORES)], axis=0)
